# revision 25
# baseline (speedup 1.0000x reference)
"""Trainium2 Bass kernel for nn_Attention (B=4, N=2048, dim=1024, 16 heads).

Sharding: each of the 8 cores handles one (batch, head-group) pair —
batch b = core//2, head-group g = core%2 (8 heads each). Per core:
  qkv part  : Q^T,K^T = W_{q,k}[g] @ x_b^T (d-major, per-tb tiles), V natural
              (+ a ones column per head for the softmax1 denominator)
  attention : S^T = K^T-slice.T @ Q^T-slice (k on partitions; the two heads
              of a pair run as row-group-tiled concurrent K=64 matmuls),
              P^T = exp(S^T * scale), O^T[d,q] (+den row) = [V|1].T @ P^T
              accumulated over k chunks
  normalize : recip = 1/(1+den) on DVE, partition-broadcast via a tiny
              K=2 matmul (ones2.T @ recip -> PSUM), multiply into hT
  proj      : OUT^T[ot,tb] = sum_hc Wp[hc].T-slices @ H^T[hc]  (full
              contraction accumulated in one PSUM group, bf16 out)
Host side: per batch, out[b] = (OUT^T_{2b} + OUT^T_{2b+1}).T + proj_b.

Matmuls run in bf16 (inputs cast host-side / in the PSUM->SBUF copies),
accumulation in fp32 PSUM; softmax denominator and normalization in fp32
internally (bf16 storage).
"""

import numpy as np
import ml_dtypes
from contextlib import ExitStack

import concourse.bass as bass
import concourse.tile as tile
from concourse import mybir
from concourse.bass_utils import run_bass_kernel_spmd

BF16 = mybir.dt.bfloat16
F32 = mybir.dt.float32
AF = mybir.ActivationFunctionType
NPBF16 = ml_dtypes.bfloat16

N_CORES = 8
B = 4
N = 2048          # tokens per batch
C = 1024          # model dim
NH = 8            # heads per core
HD = 64           # head dim
DQ = NH * HD      # q/k/v dims per core (512)
SCALE = HD ** -0.5
CC = C // 128     # contraction chunks (8)
TC = N // 128     # token chunks (16)
QB = N // 512     # q blocks of 512 (4)
KC = N // 128     # k chunks of 128 (16)
HC = DQ // 128    # head-dim chunks (4)
OT = C // 128     # output row tiles (8)
_MAX_WAITS = 1


def _split_excess_waits(nc):
    """This walrus build rejects >1 semaphore wait per instruction
    ("Too many sync wait commands"); move the excess onto NOPs inserted
    immediately before the offending instruction on the same engine."""
    n_new = 0
    for f in nc.m.functions:
        for bb in f.blocks:
            insts = bb.instructions
            i = 0
            while i < len(insts):
                inst = insts[i]
                si = inst.sync_info
                if si is not None and si.on_wait and len(si.on_wait) > _MAX_WAITS:
                    waits = list(si.on_wait)
                    keep, rest = waits[:_MAX_WAITS], waits[_MAX_WAITS:]
                    nops = []
                    while rest:
                        chunk, rest = rest[:_MAX_WAITS], rest[_MAX_WAITS:]
                        nop = mybir.InstNoOp(
                            name=f"wait-split-{n_new}", ins=[], outs=[])
                        n_new += 1
                        nop.engine = inst.engine
                        nop.sync_info = mybir.SyncInfo(on_wait=chunk, on_update=[])
                        nops.append(nop)
                    inst.sync_info = mybir.SyncInfo(
                        on_wait=keep, on_update=list(si.on_update or []))
                    for j, nop in enumerate(nops):
                        insts.insert(i + j, nop)
                    i += len(nops)
                i += 1
    return n_new


def _build(ctx: ExitStack, tc: tile.TileContext, xT, wqT, wkT, wvT, wpT, outP):
    nc = tc.nc

    persist = ctx.enter_context(tc.tile_pool(name="persist", bufs=1))
    p_pool = ctx.enter_context(tc.tile_pool(name="p", bufs=12))
    stg_pool = ctx.enter_context(tc.tile_pool(name="stg", bufs=8))
    dd_pool = ctx.enter_context(tc.tile_pool(name="dd", bufs=4))
    dram_pool = ctx.enter_context(tc.tile_pool(name="scr", bufs=4, space="DRAM"))
    acc = ctx.enter_context(tc.tile_pool(name="acc", bufs=2, space="PSUM"))
    opair = ctx.enter_context(tc.tile_pool(name="opair", bufs=2, space="PSUM"))
    sc_pool = ctx.enter_context(tc.tile_pool(name="sc", bufs=2, space="PSUM"))

    wq = persist.tile([128, CC, DQ], BF16, tag="wq")
    wk = persist.tile([128, CC, DQ], BF16, tag="wk")
    wv = persist.tile([128, CC, DQ], BF16, tag="wv")
    wp = persist.tile([128, HC, C], BF16, tag="wp")

    # per-(hc, tb) q/k tiles: scores for (hc, qb, kc) depend only on the
    # exact q/k blocks they read, so the first exp can start ~11us in
    qT4 = [[persist.tile([128, 512], BF16, tag=f"qT{h}_{t}",
                         name=f"qT{h}_{t}") for t in range(QB)]
           for h in range(HC)]
    kT4 = [[persist.tile([128, 512], BF16, tag=f"kT{h}_{t}",
                         name=f"kT{h}_{t}") for t in range(QB)]
           for h in range(HC)]
    vs = [persist.tile([128, NH * (HD + 1)], BF16, tag=f"v{i}", name=f"v{i}")
          for i in range(TC)]
    hT = persist.tile([128, HC, N], BF16, tag="hT")

    # warmup first: load the exp table + warm the PE HAM during the DMAs
    warm = persist.tile([128, 512], BF16, tag="warm")
    nc.vector.memset(warm[:], 0.0)
    wdum = stg_pool.tile([128, 128], BF16, tag="stg", name="wdum")
    nc.scalar.activation(out=wdum[:], in_=warm[:, 0:128], func=AF.Exp,
                         scale=SCALE)
    for _ in range(14):
        wps = acc.tile([128, 512], F32, tag="acc", name="warmps")
        nc.tensor.matmul(wps[:], warm[:, 0:128], warm[:], start=True,
                         stop=True)

    # ones columns for the softmax1 denominator: only column 64 of each
    # head slot needs the 1.0 (V copies fill the rest)
    for v_t in vs:
        nc.vector.memset(
            v_t[:].rearrange("p (h e) -> p h e", e=HD + 1)[:, :, HD:HD + 1],
            1.0)

    # selector rows for the den partition-broadcast matmuls (partition 64,
    # where the AV ones-column lands each head's denominator):
    # cols 0:128 -> head A selector (out rows 0-63), 128:256 -> head B
    ones2 = persist.tile([128, 256], BF16, tag="ones2")
    nc.vector.memset(ones2[64:65, :], 0.0)
    nc.vector.memset(ones2[64:65, 0:64], 1.0)
    nc.vector.memset(ones2[64:65, 192:256], 1.0)

    xt_pool = ctx.enter_context(tc.tile_pool(name="xt", bufs=1))
    xts = [xt_pool.tile([128, N], BF16, tag=f"xt{i}", name=f"xt{i}")
           for i in range(CC)]

    # input DMAs split over three queues so the startup isn't serialized
    # on one HWDGE ring: Sync takes wq+wv+x23, Scalar (idle until the
    # first exp) takes wk+wp, GpSimd SWDGE takes x01
    xT_r = xT.ap().rearrange("(cc p) t -> p cc t", p=128)
    wq_r = wqT.ap().rearrange("(cc p) d -> p cc d", p=128)
    wk_r = wkT.ap().rearrange("(cc p) d -> p cc d", p=128)
    wv_r = wvT.ap().rearrange("(cc p) d -> p cc d", p=128)
    for cc in range(CC):
        nc.sync.dma_start(out=wq[:, cc, :], in_=wq_r[:, cc, :])
        nc.scalar.dma_start(out=wk[:, cc, :], in_=wk_r[:, cc, :])
        nc.gpsimd.dma_start(out=xts[cc][:, 0:512], in_=xT_r[:, cc, 0:512])
    for cc in range(CC):
        nc.sync.dma_start(out=wv[:, cc, :], in_=wv_r[:, cc, :])
        nc.gpsimd.dma_start(out=xts[cc][:, 512:1024], in_=xT_r[:, cc, 512:1024])
    for cc in range(CC):
        nc.sync.dma_start(out=xts[cc][:, 1024:1536], in_=xT_r[:, cc, 1024:1536])
        nc.gpsimd.dma_start(out=xts[cc][:, 1536:2048],
                            in_=xT_r[:, cc, 1536:2048])
    nc.gpsimd.dma_start(
        out=wp[:], in_=wpT.ap().rearrange("(hc p) o -> p hc o", p=128))

    def qk_item(hc, tb, which):
        w_sb, dst = (wq, qT4[hc]) if which == "q" else (wk, kT4[hc])

        def item():
            ps = acc.tile([128, 512], F32, tag="acc", name="qkps")
            for cc in range(CC):
                nc.tensor.matmul(
                    ps[:],
                    w_sb[:, cc, hc * 128:(hc + 1) * 128],
                    xts[cc][:, tb * 512:(tb + 1) * 512],
                    start=(cc == 0), stop=(cc == CC - 1))
            nc.vector.tensor_copy(dst[tb][:], ps[:])
        return item

    def emit_v(t0, t1):
        for tci in range(t0, t1):
            ps = acc.tile([128, 512], F32, tag="acc")
            for cc in range(CC):
                nc.tensor.matmul(
                    ps[:],
                    xts[cc][:, tci * 128:(tci + 1) * 128],
                    wv[:, cc, :],
                    start=(cc == 0), stop=(cc == CC - 1))
            nc.vector.tensor_copy(
                vs[tci][:].rearrange("p (h e) -> p h e", e=HD + 1)[:, :, 0:HD],
                ps[:].rearrange("p (h e) -> p h e", e=HD))

    # per-unit (hc, qb) leftovers for the deferred normalize
    o_saved = {}

    def emit_attention(hc, v_filler=False, fillers=None, unit_tail=None):
        """Both heads of pair hc, interleaved per k-chunk so their K=64
        score matmuls occupy disjoint PE row groups (rows 0-63 / 64-127)
        and run concurrently in the array. `fillers[qb]` is a queue of
        small emission items (QKV work, normalizes, projection groups)
        woven between k-chunks so the PE always has independent work
        while the exp stream paces the kernel."""
        vcols = [(2 * hc + hp) * (HD + 1) for hp in range(2)]
        for qb in range(QB):
            if unit_tail is not None:
                unit_tail(qb)
            queue = list(fillers.get(qb, ())) if fillers else []
            qs = slice(qb * 512, (qb + 1) * 512)
            o_ps = [opair.tile([128, 512], F32, tag="opair", name=f"ops{hp}")
                    for hp in range(2)]
            def av(kc, hp, p_tile):
                nc.tensor.matmul(
                    o_ps[hp][0:HD + 1, :],
                    vs[kc][:, vcols[hp]:vcols[hp] + HD + 1],
                    p_tile[:, hp, :],
                    start=(kc == 0), stop=(kc == KC - 1))

            for kc in range(KC):
                # scores + exp first: they pace the kernel, and emission
                # order sets scheduling priority — fillers must lose ties
                s_ps = sc_pool.tile([128, 2, 512], F32, tag="sc")
                for hp in range(2):
                    ho = hp * 64
                    nc.tensor.matmul(
                        s_ps[:, hp, :],
                        kT4[hc][kc // 4][ho:ho + 64,
                                         (kc % 4) * 128:(kc % 4 + 1) * 128],
                        qT4[hc][qb][ho:ho + 64, :],
                        start=True, stop=True)
                p_sb = p_pool.tile([128, 2, 512], BF16, tag="p")
                nc.scalar.activation(
                    out=p_sb[:], in_=s_ps[:], func=AF.Exp, scale=SCALE)
                if v_filler and qb == 0:
                    # generate V tile kc before its first consumer below
                    emit_v(kc, kc + 1)
                if queue:
                    queue.pop(0)()
                for hp in range(2):
                    av(kc, hp, p_sb)
            # drain the pair to SBUF so the PSUM banks free immediately:
            # stgX rows 0:64 = O, row 64 = den; head B's O rows shift to
            # hT rows 64:128 via DMA
            stgA = stg_pool.tile([128, 512], BF16, tag="stg", name="stgA")
            stgB = stg_pool.tile([128, 512], BF16, tag="stg", name="stgB")
            nc.vector.tensor_copy(stgA[0:65, :], o_ps[0][0:65, :])
            nc.vector.tensor_copy(stgB[0:65, :], o_ps[1][0:65, :])
            nc.sync.dma_start(out=hT[64:128, hc, qs], in_=stgB[0:64, :])
            o_saved[(hc, qb)] = (stgA, stgB)

    def norm_qb(hc, qb):
        # recip = 1/(1+den) for one unit, computed on a [128, 8]
        # partition-major reshape (the DVE reciprocal runs 8 iterations
        # per FREE-dim element, so FD must be small), then partition-
        # broadcast via two accumulating K=1 matmuls into PSUM and
        # multiply O^T into hT
        qs = slice(qb * 512, (qb + 1) * 512)
        stgA, stgB = o_saved.pop((hc, qb))
        dn = dram_pool.tile([2, 512], BF16, tag="dn", name="dn")
        dn2 = dram_pool.tile([2, 512], BF16, tag="dn2", name="dn2")
        nc.gpsimd.dma_start(out=dn[0:1, :], in_=stgA[64:65, :])
        nc.gpsimd.dma_start(out=dn[1:2, :], in_=stgB[64:65, :])
        rcp = dd_pool.tile([128, 8], F32, tag="rcp", name="rcp")
        nc.gpsimd.dma_start(
            out=rcp[:], in_=dn[:, :].rearrange("h (a i) -> (h a) i", i=8))
        with nc.allow_low_precision(reason="bf16 softmax1 denom (tol 2e-2)"):
            nc.vector.tensor_scalar_add(rcp[:], rcp[:], 1.0)
            rcp2 = dd_pool.tile([128, 8], BF16, tag="rcp", name="rcp2")
            nc.vector.reciprocal(rcp2[:], rcp[:])
        nc.gpsimd.dma_start(
            out=dn2[:, :].rearrange("h (a i) -> (h a) i", i=8), in_=rcp2[:])
        ddA = dd_pool.tile([128, 512], BF16, tag="dd", name="ddA")
        ddB = dd_pool.tile([128, 512], BF16, tag="dd", name="ddB")
        nc.gpsimd.dma_start(out=ddA[64:65, :], in_=dn2[0:1, :])
        nc.gpsimd.dma_start(out=ddB[64:65, :], in_=dn2[1:2, :])
        rb = acc.tile([128, 512], F32, tag="acc", name="rb")
        nc.tensor.matmul(rb[:], ones2[64:65, 0:128], ddA[64:65, :],
                         start=True, stop=False)
        nc.tensor.matmul(rb[:], ones2[64:65, 128:256], ddB[64:65, :],
                         start=False, stop=True)
        nc.vector.tensor_mul(hT[0:64, hc, qs], stgA[0:64, :], rb[0:64, :])
        nc.vector.tensor_mul(hT[64:128, hc, qs], hT[64:128, hc, qs],
                             rb[64:128, :])

    def proj_items(tb):
        # full projection for token block tb: OUT^T = sum_hc Wp[hc] @ H^T[hc]
        # accumulated in one PSUM group (needs all pairs normalized at tb)
        outP_r = outP.ap().rearrange("(ot p) t -> p ot t", p=128)
        for ot in range(OT):
            def item(ot=ot, tb=tb):
                ps = acc.tile([128, 512], F32, tag="acc", name="prps")
                for hc in range(HC):
                    nc.tensor.matmul(
                        ps[:],
                        wp[:, hc, ot * 128:(ot + 1) * 128],
                        hT[:, hc, tb * 512:(tb + 1) * 512],
                        start=(hc == 0), stop=(hc == HC - 1))
                so = stg_pool.tile([128, 512], BF16, tag="stg", name="so")
                nc.vector.tensor_copy(so[:], ps[:])
                nc.sync.dma_start(
                    out=outP_r[:, ot, tb * 512:(tb + 1) * 512], in_=so[:])
            yield item

    def norm_items(hc):
        for qb in range(QB):
            yield lambda qb=qb: norm_qb(hc, qb)

    def pair_fillers(hc):
        # ordered so every item completes before its first consumer: keys
        # for k-chunks 4/8/12 early in qb0, Q(tb) one unit ahead, this
        # pair's normalizes one unit behind, next pair's tb0 Q/K late
        f = {
            0: [qk_item(hc, 1, "k"), qk_item(hc, 2, "k"),
                qk_item(hc, 3, "k"), qk_item(hc, 1, "q")],
            1: [qk_item(hc, 2, "q"), lambda: norm_qb(hc, 0)],
            2: [qk_item(hc, 3, "q"), lambda: norm_qb(hc, 1)],
            3: [lambda: norm_qb(hc, 2)],
        }
        if hc > 0:
            f[0].append(lambda: norm_qb(hc - 1, 3))
        if hc < HC - 1:
            f[3] += [qk_item(hc + 1, 0, "q"), qk_item(hc + 1, 0, "k")]
        if hc == HC - 1:
            f[2] += list(proj_items(0))
            f[3] += list(proj_items(1))
        return f

    qk_item(0, 0, "q")()
    qk_item(0, 0, "k")()
    emit_attention(0, v_filler=True, fillers=pair_fillers(0))
    emit_attention(1, fillers=pair_fillers(1))
    emit_attention(2, fillers=pair_fillers(2))
    emit_attention(3, fillers=pair_fillers(3))
    norm_qb(3, 3)
    for tb in (2, 3):
        for item in proj_items(tb):
            item()


_CACHED = None


def _get_nc():
    global _CACHED
    if _CACHED is None:
        nc = bass.Bass("TRN2", target_bir_lowering=False, debug=False)
        xT = nc.dram_tensor("xT", [C, N], BF16, kind="ExternalInput")
        wqT = nc.dram_tensor("wqT", [C, DQ], BF16, kind="ExternalInput")
        wkT = nc.dram_tensor("wkT", [C, DQ], BF16, kind="ExternalInput")
        wvT = nc.dram_tensor("wvT", [C, DQ], BF16, kind="ExternalInput")
        wpT = nc.dram_tensor("wpT", [DQ, C], BF16, kind="ExternalInput")
        outP = nc.dram_tensor("outP", [C, N], BF16, kind="ExternalOutput")
        with tile.TileContext(nc) as tc:
            with ExitStack() as ctx:
                _build(ctx, tc, xT, wqT, wkT, wvT, wpT, outP)
        _split_excess_waits(nc)
        _CACHED = nc
    return _CACHED


def run(x, mask, qkv_w, proj_w, proj_b, trace=False):
    x = np.asarray(x, dtype=np.float32)
    qkv_w = np.asarray(qkv_w, dtype=np.float32)
    proj_w = np.asarray(proj_w, dtype=np.float32)
    proj_b = np.asarray(proj_b, dtype=np.float32)

    in_maps = []
    for core in range(N_CORES):
        b, g = core // 2, core % 2
        r = slice(512 * g, 512 * g + 512)
        in_maps.append({
            "xT": np.ascontiguousarray(x[b].T).astype(NPBF16),
            "wqT": np.ascontiguousarray(qkv_w[r].T).astype(NPBF16),
            "wkT": np.ascontiguousarray(qkv_w[1024:][r].T).astype(NPBF16),
            "wvT": np.ascontiguousarray(qkv_w[2048:][r].T).astype(NPBF16),
            "wpT": np.ascontiguousarray(proj_w[:, r].T).astype(NPBF16),
        })

    nc = _get_nc()
    res = run_bass_kernel_spmd(
        nc, in_maps, core_ids=list(range(N_CORES)), trace=trace)

    out = np.empty((B, N, C), dtype=np.float32)
    for b in range(B):
        acc_np = (res.results[2 * b]["outP"].astype(np.float32)
                  + res.results[2 * b + 1]["outP"].astype(np.float32))
        out[b] = acc_np.T + proj_b
    return out, res


def kernel(x, mask, qkv_w, proj_w, proj_b):
    out, _ = run(x, mask, qkv_w, proj_w, proj_b, trace=False)
    return out
